# revision 16
# baseline (speedup 1.0000x reference)
"""Trainium2 Bass kernel for HGConv (hypergraph conv) message passing.

Contract: kernel(**inputs) takes FULL unsharded inputs (see shapes below),
shards batch b across 8 NeuronCores (data-parallel, one batch element per
core), runs a Bass/Tile kernel via run_bass_kernel_spmd, and returns the
full (8, 16) logits.

Math (per batch element), exploiting matmul associativity. With
Wfold = fc_w @ ec_proj_w (C x D), the final logits are linear in the
pooled edge features, so the C=16-wide projection can be folded in
*before* pooling:

    agg  = inc^T @ nf                      # (D, E) transposed on-chip
    es   = Wa @ agg                        # == (inc^T (nf Wa^T))^T
    attn = softmax_e(es)
    x    = agg * attn
    eft  = alpha * edge_feats^T            # built during the main loop
    Gex  = [Wfold @ eft; att_w . eft]      # (C+1, E)   mid-loop matmuls
    Hex  = Gex + [K @ x]                   # K = [(1-a)Wfold Wp; (1-a)Wp^T att_w]
    # rows 0..C-1 of Hex = Wfold @ ef_blend; row C = edge attention scores s
    a    = exp(s) / sum_e exp(s)
    logits[c] = sum_e Hex[c, e] * a[e] + bfold[c]

Ht is built TRANSPOSED ([e-part, 8, C+2] PSUM, 576 B/partition), so s is a
thin column: exp(s), the softmax sum, and the final pooling are all tiny
ops (PE matmuls / [128,8] activations) instead of E-wide vector work. The
post-DMA tail is just: copy agg -> es -> softmax -> x -> tiny K-matmuls ->
exp -> 8 pooling matmuls. Everything heavy overlaps the inc/nf DMA stream,
which is the roofline (21 MiB/pass at ~360 GB/s/core = ~61 us); across
reps the tail is fully hidden by DMA prefetch (measured ~62 us/pass).

inc streams on the Pool SWDGE queue; nf on the Activation HWDGE queue
(declared f32r in DRAM so no cast is needed); ef + weights on SP.

On-chip layout is transposed: (d on partitions, e in free dim) so every
softmax reduction over e is a free-dim reduction.
"""

import numpy as np

import concourse.mybir as mybir
import concourse.tile as tile
from concourse import bacc
from concourse.bass_utils import run_bass_kernel_spmd
from concourse.masks import make_identity

B, M, E, D, C = 8, 4096, 1024, 256, 16
C1 = C + 2  # f32r matmul moving free size must be even: pad 17 -> 18
F32 = mybir.dt.float32
F32R = mybir.dt.float32r  # full-rate matmul mode for 4-byte floats

SUBS = 4           # max 128-row subchunks per superchunk tile


def _kernel_body(tc, aps, alpha: float, ctx, reps: int = 1):
    nc = tc.nc
    nf_d, inc_d, ef_d, waT_d, wgT_d, kT_d, bf_d, out_d = aps

    consts = ctx.enter_context(tc.tile_pool(name="consts", bufs=1))
    inc_pool = ctx.enter_context(tc.tile_pool(name="inc", bufs=3))
    nf_pool = ctx.enter_context(tc.tile_pool(name="nf", bufs=3))
    sb = ctx.enter_context(tc.tile_pool(name="sb", bufs=1))
    ps_agg = ctx.enter_context(tc.tile_pool(name="ps_agg", bufs=1, space="PSUM"))
    ps_tp = ctx.enter_context(tc.tile_pool(name="ps_tp", bufs=2, space="PSUM"))

    # ---- constants / weights (HWDGE + DVE cast: keep Pool free for streams) ----
    waT_f = consts.tile([128, 2, D], F32, tag="waTf")
    nc.sync.dma_start(waT_f[:], waT_d.rearrange("(c p) j -> p c j", p=128))
    waT_sb = consts.tile([128, 2, D], F32R, tag="waT")
    nc.vector.tensor_copy(waT_sb[:], waT_f[:])
    wgT_f = consts.tile([128, 2, C1], F32, tag="wgTf")
    nc.sync.dma_start(wgT_f[:], wgT_d.rearrange("(c p) j -> p c j", p=128))
    wgT_sb = consts.tile([128, 2, C1], F32R, tag="wgT")
    nc.vector.tensor_copy(wgT_sb[:], wgT_f[:])
    kT_f = consts.tile([128, 2, C1], F32, tag="kTf")
    nc.sync.dma_start(kT_f[:], kT_d.rearrange("(c p) j -> p c j", p=128))
    kT_sb = consts.tile([128, 2, C1], F32R, tag="kT")
    nc.vector.tensor_copy(kT_sb[:], kT_f[:])
    bf_sb = consts.tile([1, C], F32, tag="bf")
    nc.sync.dma_start(bf_sb[:], bf_d[:])
    ident = consts.tile([128, 128], F32, tag="ident")
    make_identity(nc, ident[:])
    onec_f = consts.tile([128, 1], F32, tag="onecf")
    nc.gpsimd.memset(onec_f[:], 1.0)

    for _rep in range(reps):
        _one_pass(tc, aps, alpha, consts, inc_pool, nf_pool, sb, ps_agg, ps_tp,
                  waT_sb, wgT_sb, kT_sb, bf_sb, ident, onec_f)


def _one_pass(tc, aps, alpha, consts, inc_pool, nf_pool, sb, ps_agg, ps_tp,
              waT_sb, wgT_sb, kT_sb, bf_sb, ident, onec_f):
    nc = tc.nc
    nf_d, inc_d, ef_d, waT_d, wgT_d, kT_d, bf_d, out_d = aps
    # ---- edge_feats load; transposes are interleaved into the main loop ----
    ef_nat = sb.tile([128, 8, D], F32, tag="ef_nat")
    nc.sync.dma_start(ef_nat[:], ef_d.rearrange("(t p) d -> p t d", p=128))
    eft_sb = [sb.tile([128, E], F32R, tag=f"eft{di}", name=f"eft{di}") for di in range(2)]

    ALPHA = alpha

    def transpose_step(t):
        for di in range(2):
            tp = ps_tp.tile([128, 128], F32, tag="tp", name="tp")
            nc.tensor.transpose(tp[:], ef_nat[:, t, di * 128:(di + 1) * 128], ident[:])
            nc.vector.tensor_scalar_mul(eft_sb[di][:, t * 128:(t + 1) * 128], tp[:], ALPHA)

    # Ht[e, c] = (Wg . eft)^T, built TRANSPOSED (e on partitions, C+1 free) so
    # the edge-attention scores s = Ht[:, :, C] are a thin column: exp / sum /
    # pooling all become tiny ops. Emitted mid-loop (deps ready after
    # transpose_step(7)); the K @ x matmuls in the tail finish the group.
    ht_ps = ps_agg.tile([128, 8, C1], F32, tag="ht", name="ht")

    # PSUM has_written-bit semantics: ONE start=True (first matmul into the
    # bank) clears the whole zero region; every later matmul uses start=False,
    # which per-element overwrites-if-unwritten / accumulates-if-written. So
    # the 8 interleaved ec sub-regions form a single group: start on
    # (ec0, dk0) here, stop on the last K@x matmul in the tail.
    def ht_step():
        for ec in range(8):
            ecs = slice(ec * 128, (ec + 1) * 128)
            for dk in range(2):
                nc.tensor.matmul(ht_ps[:, ec, :], eft_sb[dk][:, ecs],
                                 wgT_sb[:, dk, :],
                                 start=ec == 0 and dk == 0, stop=False)

    # ---- big matmul: agg_T[d, e] = sum_m nf[m, d] * inc[m, e] ----
    # graded superchunks: big 2MiB DMAs up front, small ones at the end so
    # the last DMA->compute latency is short. inc owns the Pool SWDGE queue;
    # nf rides the Activation HWDGE queue so the two streams overlap.
    CH = [2, 4, 4, 4, 4, 4, 4, 2, 2, 2]
    agg_ps = [ps_agg.tile([128, E], F32, tag=f"pbig{di}", name=f"agg{di}") for di in range(2)]
    m0 = 0
    for s, subs in enumerate(CH):
        rows = slice(m0 * 128, (m0 + subs) * 128)
        nf_t = nf_pool.tile([128, SUBS, D], F32R, tag="nf_t")
        nc.scalar.dma_start(nf_t[:, :subs], nf_d[rows, :].rearrange("(c p) d -> p c d", p=128))
        inc_t = inc_pool.tile([128, SUBS, E], F32R, tag="inc_t")
        inc_eng = nc.gpsimd if s % 2 == 0 else nc.sync
        inc_eng.dma_start(inc_t[:, :subs], inc_d[rows, :].rearrange("(c p) e -> p c e", p=128))
        for c in range(SUBS):
            if c >= subs:
                continue
            first = m0 + c == 0
            last = m0 + c == M // 128 - 1
            for di in range(2):
                lhsT = nf_t[:, c, di * 128:(di + 1) * 128]
                for eh in range(2):
                    nc.tensor.matmul(
                        agg_ps[di][:, eh * 512:(eh + 1) * 512],
                        lhsT,
                        inc_t[:, c, eh * 512:(eh + 1) * 512],
                        start=first,
                        stop=last,
                    )
        m0 += subs
        if s < 8:
            transpose_step(s)
        if s == 8:
            ht_step()

    # ---- tail: copy agg to SBUF (frees PSUM banks for es) ----
    agg_sb = [sb.tile([128, E], F32R, tag=f"agg_sb{di}", name=f"agg_sb{di}") for di in range(2)]
    for eh in range(2):
        ehs = slice(eh * 512, (eh + 1) * 512)
        nc.vector.tensor_copy(agg_sb[0][:, ehs], agg_ps[0][:, ehs])
        nc.scalar.mul(agg_sb[1][:, ehs], agg_ps[1][:, ehs], 1.0)

    # ---- edge scores: es_T[d', e] = sum_d Wa[d', d] * agg_T[d, e] ----
    es_ps = [ps_agg.tile([128, E], F32, tag=f"pbig{di}", name=f"es{di}") for di in range(2)]
    for di in range(2):
        for dk in range(2):
            lhsT = waT_sb[:, dk, di * 128:(di + 1) * 128]
            for eh in range(2):
                nc.tensor.matmul(
                    es_ps[di][:, eh * 512:(eh + 1) * 512],
                    lhsT,
                    agg_sb[dk][:, eh * 512:(eh + 1) * 512],
                    start=dk == 0,
                    stop=dk == 1,
                )

    # ---- softmax over e (free dim) + X = agg * attn ----
    x_sb = []
    for di in range(2):
        nmax = sb.tile([128, 1], F32, tag=f"nmax{di}")
        nc.vector.tensor_reduce(nmax[:], es_ps[di][:], axis=mybir.AxisListType.X,
                                op=mybir.AluOpType.max, negate=True)
        expt = sb.tile([128, E], F32, tag=f"exp{di}")
        rsum = sb.tile([128, 1], F32, tag=f"rsum{di}")
        nc.scalar.activation(expt[:], es_ps[di][:],
                             mybir.ActivationFunctionType.Exp,
                             bias=nmax[:], accum_out=rsum[:])
        rinv = sb.tile([128, 1], F32, tag=f"rinv{di}")
        nc.vector.reciprocal(rinv[:], rsum[:])
        xt = sb.tile([128, E], F32R, tag=f"x{di}")
        # X = (exp * rinv) * agg  (normalized attention times aggregate)
        nc.vector.scalar_tensor_tensor(xt[:], expt[:], rinv[:], agg_sb[di][:],
                                       op0=mybir.AluOpType.mult,
                                       op1=mybir.AluOpType.mult)
        x_sb.append(xt)

    # ---- Ht += (K @ x)^T  (continues the ht accumulation group) ----
    for ec in range(8):
        ecs = slice(ec * 128, (ec + 1) * 128)
        for dk in range(2):
            nc.tensor.matmul(ht_ps[:, ec, :], x_sb[dk][:, ecs], kT_sb[:, dk, :],
                             start=False, stop=ec == 7 and dk == 1)

    # ---- Ht -> SBUF; a = exp(s) with s = Ht[:, :, C]  (|s|<=~3, no max) ----
    ht_sb = sb.tile([128, 8, C1], F32R, tag="ht_sb")
    nc.vector.tensor_copy(ht_sb[:], ht_ps[:])
    aT = sb.tile([128, 8], F32R, tag="aT")
    acc = sb.tile([128, 1], F32, tag="acc")
    nc.scalar.activation(aT[:], ht_sb[:, :, C], mybir.ActivationFunctionType.Exp,
                         accum_out=acc[:])
    # total sum over e: contract acc over partitions via K=128 matmul
    ss_ps = ps_tp.tile([1, 1], F32, tag="tp", name="ss_ps")
    nc.tensor.matmul(ss_ps[:], acc[:], onec_f[:], start=True, stop=True)
    sinv = sb.tile([1, 1], F32, tag="sinv")
    nc.vector.reciprocal(sinv[:], ss_ps[:])

    # ---- logits[c] = sum_e exp(s)[e] Ht[e, c] / ssum + bfold[c] ----
    lg_ps = ps_tp.tile([1, C1], F32, tag="tp", name="lg_ps")
    for ec in range(8):
        nc.tensor.matmul(lg_ps[:], aT[:, ec:ec + 1], ht_sb[:, ec, :],
                         start=ec == 0, stop=ec == 7)
    lg_sb = sb.tile([1, C], F32, tag="lgsb")
    nc.vector.scalar_tensor_tensor(lg_sb[:], lg_ps[:, 0:C], sinv[:], bf_sb[:],
                                   op0=mybir.AluOpType.mult,
                                   op1=mybir.AluOpType.add)
    nc.sync.dma_start(out_d[:], lg_sb[:])


def build(alpha: float, reps: int = 1):
    nc = bacc.Bacc("TRN2", target_bir_lowering=False, debug=False)
    nf_d = nc.dram_tensor("node_feats", [M, D], F32R, kind="ExternalInput").ap()
    inc_d = nc.dram_tensor("inc_mat", [M, E], F32R, kind="ExternalInput").ap()
    ef_d = nc.dram_tensor("edge_feats", [E, D], F32, kind="ExternalInput").ap()
    waT_d = nc.dram_tensor("waT", [D, D], F32, kind="ExternalInput").ap()
    wgT_d = nc.dram_tensor("wgT", [D, C1], F32, kind="ExternalInput").ap()
    kT_d = nc.dram_tensor("kT", [D, C1], F32, kind="ExternalInput").ap()
    bf_d = nc.dram_tensor("bfold", [1, C], F32, kind="ExternalInput").ap()
    out_d = nc.dram_tensor("logits", [1, C], F32, kind="ExternalOutput").ap()
    aps = (nf_d, inc_d, ef_d, waT_d, wgT_d, kT_d, bf_d, out_d)
    from contextlib import ExitStack

    with tile.TileContext(nc) as tc, ExitStack() as ctx:
        _kernel_body(tc, aps, alpha, ctx, reps=reps)
    nc.compile()
    return nc


def make_in_maps(inputs: dict) -> list[dict]:
    nf = np.ascontiguousarray(np.asarray(inputs["node_feats"], np.float32))
    inc = np.ascontiguousarray(np.asarray(inputs["inc_mat"], np.float32))
    ef = np.ascontiguousarray(np.asarray(inputs["edge_feats"], np.float32))
    Wa = np.asarray(inputs["Wa"], np.float32)
    Wp = np.asarray(inputs["Wp"], np.float32)
    alpha = float(np.asarray(inputs["alpha"]))
    att = np.asarray(inputs["ec_att_w"], np.float32).reshape(1, D)
    ec_w = np.asarray(inputs["ec_proj_w"], np.float32)
    ec_b = np.asarray(inputs["ec_proj_b"], np.float32)
    fc_w = np.asarray(inputs["fc_w"], np.float32)
    fc_b = np.asarray(inputs["fc_b"], np.float32)

    waT = np.ascontiguousarray(Wa.T)
    wfold = fc_w @ ec_w                                     # (C, D)
    pad = np.zeros((D, 1), np.float32)                      # even-size pad col
    # Gex weights: [Wfold; att_w; 0] applied to eft (= alpha * ef^T)
    wgT = np.ascontiguousarray(
        np.concatenate([wfold.T, att.T, pad], axis=1))      # (D, C+2)
    # K weights: [(1-a) Wfold Wp; (1-a) Wp^T att_w; 0] applied to x
    kT = np.ascontiguousarray(np.concatenate(
        [(1.0 - alpha) * (wfold @ Wp).T,
         (1.0 - alpha) * (att @ Wp).T, pad], axis=1))       # (D, C+2)
    bfold = np.ascontiguousarray((ec_b @ fc_w.T + fc_b).reshape(1, C))

    return [
        dict(node_feats=nf[b], inc_mat=inc[b], edge_feats=ef[b],
             waT=waT, wgT=wgT, kT=kT, bfold=bfold)
        for b in range(B)
    ]


def kernel(**inputs) -> np.ndarray:
    alpha = float(np.asarray(inputs["alpha"]))
    nc = build(alpha)
    in_maps = make_in_maps(inputs)
    res = run_bass_kernel_spmd(nc, in_maps, core_ids=list(range(B)))
    return np.stack([res.results[b]["logits"].reshape(C) for b in range(B)], axis=0)


# revision 17
# speedup vs baseline: 1.1689x; 1.1689x over previous
"""Trainium2 Bass kernel for HGConv (hypergraph conv) message passing.

Contract: kernel(**inputs) takes FULL unsharded inputs (see shapes below),
shards batch b across 8 NeuronCores (data-parallel, one batch element per
core), runs a Bass/Tile kernel via run_bass_kernel_spmd, and returns the
full (8, 16) logits.

Math (per batch element), exploiting matmul associativity. With
Wfold = fc_w @ ec_proj_w (C x D), the final logits are linear in the
pooled edge features, so the C=16-wide projection can be folded in
*before* pooling:

    agg  = inc^T @ nf                      # (D, E) transposed on-chip
    es   = Wa @ agg                        # == (inc^T (nf Wa^T))^T
    attn = softmax_e(es)
    x    = agg * attn
    eft  = alpha * edge_feats^T            # built during the main loop
    Gex  = [Wfold @ eft; att_w . eft]      # (C+1, E)   mid-loop matmuls
    Hex  = Gex + [K @ x]                   # K = [(1-a)Wfold Wp; (1-a)Wp^T att_w]
    # rows 0..C-1 of Hex = Wfold @ ef_blend; row C = edge attention scores s
    a    = exp(s) / sum_e exp(s)
    logits[c] = sum_e Hex[c, e] * a[e] + bfold[c]

Ht is built TRANSPOSED ([e-part, 8, C+2] PSUM, 576 B/partition), so s is a
thin column: exp(s), the softmax sum, and the final pooling are all tiny
ops (PE matmuls / [128,8] activations) instead of E-wide vector work. The
post-DMA tail is just: copy agg -> es -> softmax -> x -> tiny K-matmuls ->
exp -> 8 pooling matmuls. Everything heavy overlaps the inc/nf DMA stream,
which is the roofline (21 MiB/pass at ~360 GB/s/core = ~61 us); across
reps the tail is fully hidden by DMA prefetch (measured ~62 us/pass).

inc streams on the Pool SWDGE queue; nf on the Activation HWDGE queue
(declared f32r in DRAM so no cast is needed); ef + weights on SP.

On-chip layout is transposed: (d on partitions, e in free dim) so every
softmax reduction over e is a free-dim reduction.
"""

import numpy as np

import concourse.mybir as mybir
import concourse.tile as tile
from concourse import bacc
from concourse.bass_utils import run_bass_kernel_spmd
from concourse.masks import make_identity

B, M, E, D, C = 8, 4096, 1024, 256, 16
C1 = C + 2  # f32r matmul moving free size must be even: pad 17 -> 18
F32 = mybir.dt.float32
F32R = mybir.dt.float32r  # full-rate matmul mode for 4-byte floats

SUBS = 4           # max 128-row subchunks per superchunk tile


def _kernel_body(tc, aps, alpha: float, ctx, reps: int = 1):
    nc = tc.nc
    nf_d, inc_d, ef_d, waT_d, wgT_d, kT_d, bf_d, out_d = aps

    consts = ctx.enter_context(tc.tile_pool(name="consts", bufs=1))
    inc_pool = ctx.enter_context(tc.tile_pool(name="inc", bufs=3))
    nf_pool = ctx.enter_context(tc.tile_pool(name="nf", bufs=3))
    sb = ctx.enter_context(tc.tile_pool(name="sb", bufs=1))
    ps_agg = ctx.enter_context(tc.tile_pool(name="ps_agg", bufs=1, space="PSUM"))
    ps_tp = ctx.enter_context(tc.tile_pool(name="ps_tp", bufs=2, space="PSUM"))

    # ---- constants / weights (HWDGE + DVE cast: keep Pool free for streams) ----
    waT_f = consts.tile([128, 2, D], F32, tag="waTf")
    nc.sync.dma_start(waT_f[:], waT_d.rearrange("(c p) j -> p c j", p=128))
    waT_sb = consts.tile([128, 2, D], F32R, tag="waT")
    nc.vector.tensor_copy(waT_sb[:], waT_f[:])
    wgT_f = consts.tile([128, 2, C1], F32, tag="wgTf")
    nc.sync.dma_start(wgT_f[:], wgT_d.rearrange("(c p) j -> p c j", p=128))
    wgT_sb = consts.tile([128, 2, C1], F32R, tag="wgT")
    nc.vector.tensor_copy(wgT_sb[:], wgT_f[:])
    kT_f = consts.tile([128, 2, C1], F32, tag="kTf")
    nc.sync.dma_start(kT_f[:], kT_d.rearrange("(c p) j -> p c j", p=128))
    kT_sb = consts.tile([128, 2, C1], F32R, tag="kT")
    nc.vector.tensor_copy(kT_sb[:], kT_f[:])
    bf_sb = consts.tile([1, C], F32, tag="bf")
    nc.sync.dma_start(bf_sb[:], bf_d[:])
    ident = consts.tile([128, 128], F32, tag="ident")
    make_identity(nc, ident[:])
    onec_f = consts.tile([128, 1], F32, tag="onecf")
    nc.gpsimd.memset(onec_f[:], 1.0)

    for _rep in range(reps):
        _one_pass(tc, aps, alpha, consts, inc_pool, nf_pool, sb, ps_agg, ps_tp,
                  waT_sb, wgT_sb, kT_sb, bf_sb, ident, onec_f)


def _one_pass(tc, aps, alpha, consts, inc_pool, nf_pool, sb, ps_agg, ps_tp,
              waT_sb, wgT_sb, kT_sb, bf_sb, ident, onec_f):
    nc = tc.nc
    nf_d, inc_d, ef_d, waT_d, wgT_d, kT_d, bf_d, out_d = aps
    # ---- edge_feats load; transposes are interleaved into the main loop ----
    ef_nat = sb.tile([128, 8, D], F32, tag="ef_nat")
    nc.sync.dma_start(ef_nat[:], ef_d.rearrange("(t p) d -> p t d", p=128))
    eft_sb = [sb.tile([128, E], F32R, tag=f"eft{di}", name=f"eft{di}") for di in range(2)]

    ALPHA = alpha

    def transpose_step(t):
        for di in range(2):
            tp = ps_tp.tile([128, 128], F32, tag="tp", name="tp")
            nc.tensor.transpose(tp[:], ef_nat[:, t, di * 128:(di + 1) * 128], ident[:])
            nc.vector.tensor_scalar_mul(eft_sb[di][:, t * 128:(t + 1) * 128], tp[:], ALPHA)

    # Ht[e, c] = (Wg . eft)^T, built TRANSPOSED (e on partitions, C+1 free) so
    # the edge-attention scores s = Ht[:, :, C] are a thin column: exp / sum /
    # pooling all become tiny ops. Emitted mid-loop (deps ready after
    # transpose_step(7)); the K @ x matmuls in the tail finish the group.
    ht_ps = ps_agg.tile([128, 8, C1], F32, tag="ht", name="ht")

    # PSUM has_written-bit semantics: ONE start=True (first matmul into the
    # bank) clears the whole zero region; every later matmul uses start=False,
    # which per-element overwrites-if-unwritten / accumulates-if-written. So
    # the 8 interleaved ec sub-regions form a single group: start on
    # (ec0, dk0) here, stop on the last K@x matmul in the tail.
    def ht_step():
        for ec in range(8):
            ecs = slice(ec * 128, (ec + 1) * 128)
            for dk in range(2):
                nc.tensor.matmul(ht_ps[:, ec, :], eft_sb[dk][:, ecs],
                                 wgT_sb[:, dk, :],
                                 start=ec == 0 and dk == 0, stop=False)

    # ---- big matmul: agg_T[d, e] = sum_m nf[m, d] * inc[m, e] ----
    # graded superchunks: big 2MiB DMAs up front, small ones at the end so
    # the last DMA->compute latency is short. inc owns the Pool SWDGE queue;
    # nf rides the Activation HWDGE queue so the two streams overlap.
    CH = [2, 4, 4, 4, 4, 4, 4, 2, 2, 2]
    agg_ps = [ps_agg.tile([128, E], F32, tag=f"pbig{di}", name=f"agg{di}") for di in range(2)]
    m0 = 0
    for s, subs in enumerate(CH):
        rows = slice(m0 * 128, (m0 + subs) * 128)
        nf_t = nf_pool.tile([128, SUBS, D], F32R, tag="nf_t")
        nc.scalar.dma_start(nf_t[:, :subs], nf_d[rows, :].rearrange("(c p) d -> p c d", p=128))
        inc_t = inc_pool.tile([128, SUBS, E], F32R, tag="inc_t")
        nc.gpsimd.dma_start(inc_t[:, :subs], inc_d[rows, :].rearrange("(c p) e -> p c e", p=128))
        for c in range(SUBS):
            if c >= subs:
                continue
            first = m0 + c == 0
            last = m0 + c == M // 128 - 1
            for di in range(2):
                lhsT = nf_t[:, c, di * 128:(di + 1) * 128]
                for eh in range(2):
                    nc.tensor.matmul(
                        agg_ps[di][:, eh * 512:(eh + 1) * 512],
                        lhsT,
                        inc_t[:, c, eh * 512:(eh + 1) * 512],
                        start=first,
                        stop=last,
                    )
        m0 += subs
        if s < 8:
            transpose_step(s)
        if s == 8:
            ht_step()

    # ---- tail: copy agg to SBUF (frees PSUM banks for es) ----
    agg_sb = [sb.tile([128, E], F32R, tag=f"agg_sb{di}", name=f"agg_sb{di}") for di in range(2)]
    for eh in range(2):
        ehs = slice(eh * 512, (eh + 1) * 512)
        nc.vector.tensor_copy(agg_sb[0][:, ehs], agg_ps[0][:, ehs])
        nc.scalar.mul(agg_sb[1][:, ehs], agg_ps[1][:, ehs], 1.0)

    # ---- edge scores: es_T[d', e] = sum_d Wa[d', d] * agg_T[d, e] ----
    es_ps = [ps_agg.tile([128, E], F32, tag=f"pbig{di}", name=f"es{di}") for di in range(2)]
    for di in range(2):
        for dk in range(2):
            lhsT = waT_sb[:, dk, di * 128:(di + 1) * 128]
            for eh in range(2):
                nc.tensor.matmul(
                    es_ps[di][:, eh * 512:(eh + 1) * 512],
                    lhsT,
                    agg_sb[dk][:, eh * 512:(eh + 1) * 512],
                    start=dk == 0,
                    stop=dk == 1,
                )

    # ---- softmax over e (free dim) + X = agg * attn ----
    x_sb = []
    for di in range(2):
        nmax = sb.tile([128, 1], F32, tag=f"nmax{di}")
        nc.vector.tensor_reduce(nmax[:], es_ps[di][:], axis=mybir.AxisListType.X,
                                op=mybir.AluOpType.max, negate=True)
        expt = sb.tile([128, E], F32, tag=f"exp{di}")
        rsum = sb.tile([128, 1], F32, tag=f"rsum{di}")
        nc.scalar.activation(expt[:], es_ps[di][:],
                             mybir.ActivationFunctionType.Exp,
                             bias=nmax[:], accum_out=rsum[:])
        rinv = sb.tile([128, 1], F32, tag=f"rinv{di}")
        nc.vector.reciprocal(rinv[:], rsum[:])
        xt = sb.tile([128, E], F32R, tag=f"x{di}")
        # X = (exp * rinv) * agg  (normalized attention times aggregate)
        nc.vector.scalar_tensor_tensor(xt[:], expt[:], rinv[:], agg_sb[di][:],
                                       op0=mybir.AluOpType.mult,
                                       op1=mybir.AluOpType.mult)
        x_sb.append(xt)

    # ---- Ht += (K @ x)^T  (continues the ht accumulation group) ----
    for ec in range(8):
        ecs = slice(ec * 128, (ec + 1) * 128)
        for dk in range(2):
            nc.tensor.matmul(ht_ps[:, ec, :], x_sb[dk][:, ecs], kT_sb[:, dk, :],
                             start=False, stop=ec == 7 and dk == 1)

    # ---- Ht -> SBUF; a = exp(s) with s = Ht[:, :, C]  (|s|<=~3, no max) ----
    ht_sb = sb.tile([128, 8, C1], F32R, tag="ht_sb")
    nc.vector.tensor_copy(ht_sb[:], ht_ps[:])
    aT = sb.tile([128, 8], F32R, tag="aT")
    acc = sb.tile([128, 1], F32, tag="acc")
    nc.scalar.activation(aT[:], ht_sb[:, :, C], mybir.ActivationFunctionType.Exp,
                         accum_out=acc[:])
    # total sum over e: contract acc over partitions via K=128 matmul
    ss_ps = ps_tp.tile([1, 1], F32, tag="tp", name="ss_ps")
    nc.tensor.matmul(ss_ps[:], acc[:], onec_f[:], start=True, stop=True)
    sinv = sb.tile([1, 1], F32, tag="sinv")
    nc.vector.reciprocal(sinv[:], ss_ps[:])

    # ---- logits[c] = sum_e exp(s)[e] Ht[e, c] / ssum + bfold[c] ----
    lg_ps = ps_tp.tile([1, C1], F32, tag="tp", name="lg_ps")
    for ec in range(8):
        nc.tensor.matmul(lg_ps[:], aT[:, ec:ec + 1], ht_sb[:, ec, :],
                         start=ec == 0, stop=ec == 7)
    lg_sb = sb.tile([1, C], F32, tag="lgsb")
    nc.vector.scalar_tensor_tensor(lg_sb[:], lg_ps[:, 0:C], sinv[:], bf_sb[:],
                                   op0=mybir.AluOpType.mult,
                                   op1=mybir.AluOpType.add)
    nc.sync.dma_start(out_d[:], lg_sb[:])


def build(alpha: float, reps: int = 1):
    nc = bacc.Bacc("TRN2", target_bir_lowering=False, debug=False)
    nf_d = nc.dram_tensor("node_feats", [M, D], F32R, kind="ExternalInput").ap()
    inc_d = nc.dram_tensor("inc_mat", [M, E], F32R, kind="ExternalInput").ap()
    ef_d = nc.dram_tensor("edge_feats", [E, D], F32, kind="ExternalInput").ap()
    waT_d = nc.dram_tensor("waT", [D, D], F32, kind="ExternalInput").ap()
    wgT_d = nc.dram_tensor("wgT", [D, C1], F32, kind="ExternalInput").ap()
    kT_d = nc.dram_tensor("kT", [D, C1], F32, kind="ExternalInput").ap()
    bf_d = nc.dram_tensor("bfold", [1, C], F32, kind="ExternalInput").ap()
    out_d = nc.dram_tensor("logits", [1, C], F32, kind="ExternalOutput").ap()
    aps = (nf_d, inc_d, ef_d, waT_d, wgT_d, kT_d, bf_d, out_d)
    from contextlib import ExitStack

    with tile.TileContext(nc) as tc, ExitStack() as ctx:
        _kernel_body(tc, aps, alpha, ctx, reps=reps)
    nc.compile()
    return nc


def make_in_maps(inputs: dict) -> list[dict]:
    nf = np.ascontiguousarray(np.asarray(inputs["node_feats"], np.float32))
    inc = np.ascontiguousarray(np.asarray(inputs["inc_mat"], np.float32))
    ef = np.ascontiguousarray(np.asarray(inputs["edge_feats"], np.float32))
    Wa = np.asarray(inputs["Wa"], np.float32)
    Wp = np.asarray(inputs["Wp"], np.float32)
    alpha = float(np.asarray(inputs["alpha"]))
    att = np.asarray(inputs["ec_att_w"], np.float32).reshape(1, D)
    ec_w = np.asarray(inputs["ec_proj_w"], np.float32)
    ec_b = np.asarray(inputs["ec_proj_b"], np.float32)
    fc_w = np.asarray(inputs["fc_w"], np.float32)
    fc_b = np.asarray(inputs["fc_b"], np.float32)

    waT = np.ascontiguousarray(Wa.T)
    wfold = fc_w @ ec_w                                     # (C, D)
    pad = np.zeros((D, 1), np.float32)                      # even-size pad col
    # Gex weights: [Wfold; att_w; 0] applied to eft (= alpha * ef^T)
    wgT = np.ascontiguousarray(
        np.concatenate([wfold.T, att.T, pad], axis=1))      # (D, C+2)
    # K weights: [(1-a) Wfold Wp; (1-a) Wp^T att_w; 0] applied to x
    kT = np.ascontiguousarray(np.concatenate(
        [(1.0 - alpha) * (wfold @ Wp).T,
         (1.0 - alpha) * (att @ Wp).T, pad], axis=1))       # (D, C+2)
    bfold = np.ascontiguousarray((ec_b @ fc_w.T + fc_b).reshape(1, C))

    return [
        dict(node_feats=nf[b], inc_mat=inc[b], edge_feats=ef[b],
             waT=waT, wgT=wgT, kT=kT, bfold=bfold)
        for b in range(B)
    ]


def kernel(**inputs) -> np.ndarray:
    alpha = float(np.asarray(inputs["alpha"]))
    nc = build(alpha)
    in_maps = make_in_maps(inputs)
    res = run_bass_kernel_spmd(nc, in_maps, core_ids=list(range(B)))
    return np.stack([res.results[b]["logits"].reshape(C) for b in range(B)], axis=0)
